# revision 49
# baseline (speedup 1.0000x reference)
"""Multi-head attention (RoPE + causal softmax) Trainium2 Bass kernel.

Problem: nn_MultiHeadAttention (B=16, S=512, D=1024, H=16, Hd=64).
Sharding: data-parallel over batch — 2 batches per core on 8 NeuronCores.
Feature-major device layout; per-core pipeline: q/k projections + RoPE,
v projection (ones-column augmented), per-head-pair causal attention with
PE-packed score matmuls, softmax via exp + ones-row denominators, Wo
projection.  Measured ~244-246us on HW.
"""

import numpy as np
import ml_dtypes

BF16 = ml_dtypes.bfloat16

B, S, D = 16, 512, 1024
H, HD = 16, 64
NCORES = 8
BPC = B // NCORES
T = BPC * S

_CACHE = {}


def _rope_tables():
    inv_freq = 1.0 / (10000.0 ** (np.arange(0, HD, 2, dtype=np.float64) / HD))
    t = np.arange(S, dtype=np.float64)
    freqs = np.outer(t, inv_freq)
    emb = np.concatenate([freqs, freqs], -1)
    return np.cos(emb), np.sin(emb)


def _host_consts():
    cos, sin = _rope_tables()
    tan = sin / cos
    cols = np.arange(T) % S
    cosT = np.ascontiguousarray(np.tile(cos[cols].T, (2, 1))).astype(BF16)
    tanT = np.ascontiguousarray(np.tile(tan[cols].T, (2, 1))).astype(BF16)
    R64 = np.zeros((64, 64), np.float32)
    R64[np.arange(32), np.arange(32) + 32] = -1.0
    R64[np.arange(32) + 32, np.arange(32)] = 1.0
    R128 = np.zeros((128, 128), np.float32)
    R128[:64, :64] = R64
    R128[64:, 64:] = R64
    RT = np.ascontiguousarray(R128.T).astype(BF16)
    mask01 = (np.arange(128)[None, :] >= np.arange(128)[:, None]).astype(BF16)
    mask2 = np.ascontiguousarray(np.concatenate([mask01, mask01], axis=1))
    return cosT, tanT, RT, mask2


def _build_bass(dump_debug=False):
    import concourse.bacc as bacc
    import concourse.tile as tile
    import concourse.mybir as mybir

    dt = mybir.dt
    f32, bf16 = dt.float32, dt.bfloat16
    Exp = mybir.ActivationFunctionType.Exp

    nc = bacc.Bacc("TRN2", target_bir_lowering=False, debug=False, enable_asserts=False)

    xT_d = nc.dram_tensor("xT", [D, T], bf16, kind="ExternalInput").ap()
    wq_d = nc.dram_tensor("WqT", [D, D], bf16, kind="ExternalInput").ap()
    wk_d = nc.dram_tensor("WkT", [D, D], bf16, kind="ExternalInput").ap()
    wv_d = nc.dram_tensor("WvT", [D, D], bf16, kind="ExternalInput").ap()
    wo_d = nc.dram_tensor("WoT", [D, D], bf16, kind="ExternalInput").ap()
    cos_d = nc.dram_tensor("cosT", [128, T], bf16, kind="ExternalInput").ap()
    tan_d = nc.dram_tensor("tanT", [128, T], bf16, kind="ExternalInput").ap()
    rt_d = nc.dram_tensor("RT", [128, 128], bf16, kind="ExternalInput").ap()
    mask_d = nc.dram_tensor("mask2", [128, 256], bf16, kind="ExternalInput").ap()
    out_d = nc.dram_tensor("outT", [D, T], f32, kind="ExternalOutput").ap()

    KC = D // 128

    with tile.TileContext(nc) as tc:
        with (
            tc.tile_pool(name="consts", bufs=1) as consts,
            tc.tile_pool(name="persist", bufs=1) as persist,
            tc.tile_pool(name="work", bufs=3) as work,
            tc.tile_pool(name="expp", bufs=3) as expp,
            tc.tile_pool(name="ps_a", bufs=4, space="PSUM") as ps_a,
            tc.tile_pool(name="ps_b", bufs=2, space="PSUM") as ps_b,
        ):
            def load(pool, dram, shape, dtyp, nm):
                t_ = pool.tile(shape, dtyp, name=nm)
                nc.sync.dma_start(out=t_, in_=dram)
                return t_

            xT = [load(consts, xT_d[k * 128:(k + 1) * 128, :], [128, T], bf16, f"xT{k}") for k in range(KC)]
            wq = [load(consts, wq_d[k * 128:(k + 1) * 128, :], [128, D], bf16, f"wq{k}") for k in range(KC)]
            RT = load(consts, rt_d, [128, 128], bf16, "RT")
            cosT = load(consts, cos_d, [128, T], bf16, "cosT")
            tanT = load(consts, tan_d, [128, T], bf16, "tanT")
            wk = [load(consts, wk_d[k * 128:(k + 1) * 128, :], [128, D], bf16, f"wk{k}") for k in range(KC)]
            wv = [load(consts, wv_d[k * 128:(k + 1) * 128, :], [128, D], bf16, f"wv{k}") for k in range(KC)]
            mask2 = load(consts, mask_d, [128, 2, 128], bf16, "mask2")
            wo = [load(consts, wo_d[k * 128:(k + 1) * 128, :], [128, D], bf16, f"wo{k}") for k in range(KC)]

            qrot = [persist.tile([128, T], bf16, name=f"qrot{m}") for m in range(KC)]
            krot = [persist.tile([128, T], bf16, name=f"krot{m}") for m in range(KC)]
            vsb = [persist.tile([128, H * 65], bf16, name=f"vsb{t_}") for t_ in range(T // 128)]
            att = [persist.tile([128, T], bf16, name=f"att{m}") for m in range(KC)]

            for t_ in range(T // 128):
                vt = vsb[t_].rearrange("p (h w) -> p h w", w=65)
                nc.gpsimd.memset(vt[:, :, 64:65], 1.0)

            # qk projection with the RoPE tan trick, split into A (projection
            # matmuls + pre2 = pp*tan on DVE) and B (rotation matmul that
            # accumulates into the same PSUM + rot = pp*cos).  Emitting B
            # after the sibling A hides the pre2 chain behind 8 matmuls.
            # Identity: R@(pp*tan)*cos == rotate_half(pp)*sin because the
            # RoPE tables repeat with period 32.
            def emit_qk_A(nb, w_sb, rot, m):
                cols = slice(nb * S, (nb + 1) * S)
                pp = ps_a.tile([128, S], f32, name="pp", tag="acc", bufs=2)
                for k in range(KC):
                    nc.tensor.matmul(
                        pp, w_sb[k][:, m * 128:(m + 1) * 128], xT[k][:, cols],
                        start=(k == 0), stop=(k == KC - 1))
                pre2 = work.tile([128, S], bf16, name="pre2", tag="pre2", bufs=2)
                nc.vector.tensor_mul(pre2, pp, tanT[:, cols])
                return (pp, pre2, rot, m, cols)

            def emit_qk_B(st):
                pp, pre2, rot, m, cols = st
                nc.tensor.matmul(pp, RT, pre2, start=False, stop=True,
                                 skip_group_check=True)
                nc.vector.tensor_mul(rot[m][:, cols], pp, cosT[:, cols])

            def emit_qk_unit(nb, m):
                st_q = emit_qk_A(nb, wq, qrot, m)
                st_k = emit_qk_A(nb, wk, krot, m)
                emit_qk_B(st_q)
                emit_qk_B(st_k)

            def emit_v_group(b, tch, nh):
                vt = vsb[tch].rearrange("p (h w) -> p h w", w=65)
                vp = ps_a.tile([128, S], f32, name="vp", tag="acc", bufs=2)
                for k in range(KC):
                    nc.tensor.matmul(
                        vp, xT[k][:, tch * 128:(tch + 1) * 128],
                        wv[k][:, nh * S:(nh + 1) * S],
                        start=(k == 0), stop=(k == KC - 1))
                nc.scalar.copy(
                    vt[:, nh * 8:(nh + 1) * 8, 0:64],
                    vp.rearrange("p (h w) -> p h w", w=64))

            def emit_attn_head(b, h, exs):
                bcols = slice(b * S, (b + 1) * S)
                mh, p0 = h // 2, (h % 2) * 64
                hi = h % 2
                av = ps_a.tile([128, S], f32, name="av", tag="av", bufs=2)
                for i in range(4):
                    lo = i * 128
                    nc.tensor.matmul(
                        av[0:65, lo:S],
                        vsb[b * 4 + i][:, h * 65: h * 65 + 65],
                        exs[i][:, hi, lo:S],
                        start=(i == 0), stop=(i == 3), skip_group_check=True)
                ss = work.tile([1, S], f32, name="ss", tag="ss")
                nc.vector.tensor_copy(ss, av[64:65, :])
                st = work.tile([128, 4], f32, name="st", tag="st")
                nc.gpsimd.dma_start(out=st, in_=ss)
                rt = work.tile([128, 4], f32, name="rt", tag="rt")
                nc.vector.reciprocal(rt, st)
                rr = work.tile([1, S], f32, name="rr", tag="rr")
                nc.gpsimd.dma_start(out=rr, in_=rt)
                rb = work.tile([64, S], f32, name="rb", tag="rb", bufs=2)
                nc.gpsimd.partition_broadcast(rb, rr)
                return (av, rb, mh, p0, bcols)

            def emit_head_mul(st):
                # deferred one pair so the rb broadcast has landed by the
                # time the DVE queue reaches this — keeps masks (which gate
                # attn@v matmuls) from queueing behind an rb wait
                av, rb, mh, p0, bcols = st
                nc.vector.tensor_mul(att[mh][p0:p0 + 64, bcols], av[0:64, :], rb)

            def emit_attn_pair(b, j, pend):
                mh = j
                exs = []
                for i in range(4):
                    lo = i * 128
                    sc = ps_b.tile([128, 2, S], f32, name="sc", tag="ps_b")
                    for hi, p0 in ((0, 0), (1, 64)):
                        nc.tensor.matmul(
                            sc[:, hi, 0:S - lo],
                            krot[mh][p0:p0 + 64, b * S + lo: b * S + lo + 128],
                            qrot[mh][p0:p0 + 64, b * S + lo: (b + 1) * S],
                            start=True, stop=True)
                    ex = expp.tile([128, 2, S], bf16, name="ex", tag=f"ex{i}")
                    nc.scalar.activation(ex[:, :, lo:S], sc[:, :, 0:S - lo], Exp, scale=0.125)
                    nc.vector.tensor_mul(ex[:, :, lo:lo + 128], ex[:, :, lo:lo + 128], mask2)
                    exs.append(ex)
                for st in pend:
                    emit_head_mul(st)
                return [emit_attn_head(b, 2 * j, exs),
                        emit_attn_head(b, 2 * j + 1, exs)]

            def emit_wo_group(b, m):
                bcols = slice(b * S, (b + 1) * S)
                fin = ps_a.tile([128, S], f32, name="fin", tag="acc", bufs=2)
                for k in range(KC):
                    nc.tensor.matmul(
                        fin, wo[k][:, m * 128:(m + 1) * 128], att[k][:, bcols],
                        start=(k == 0), stop=(k == KC - 1))
                ob = work.tile([128, S], f32, name="ob", tag="ob", bufs=2)
                nc.vector.tensor_copy(ob, fin)
                nc.sync.dma_start(out=out_d[m * 128:(m + 1) * 128, bcols], in_=ob)

            for m in range(KC):
                emit_qk_unit(0, m)
            for tch in range(4):
                for nh in range(2):
                    emit_v_group(0, tch, nh)
            v1 = [(tch, nh) for tch in range(4, 8) for nh in range(2)]
            pend = []
            for j in range(H // 2):
                pend = emit_attn_pair(0, j, pend)
                emit_qk_unit(1, j)
                emit_v_group(1, *v1[j])
            for j in range(H // 2):
                pend = emit_attn_pair(1, j, pend)
                emit_wo_group(0, j)
            for st in pend:
                emit_head_mul(st)
            for m in range(KC):
                emit_wo_group(1, m)

    nc.compile()
    return nc


def _get_nc():
    if "nc" not in _CACHE:
        _CACHE["nc"] = _build_bass()
    return _CACHE["nc"]


def make_in_maps(x, Wq, Wk, Wv, Wo):
    cosT, tanT, RT, mask2 = _host_consts()
    shared = {
        "WqT": np.ascontiguousarray(Wq.T).astype(BF16),
        "WkT": np.ascontiguousarray(Wk.T).astype(BF16),
        "WvT": np.ascontiguousarray(Wv.T).astype(BF16),
        "WoT": np.ascontiguousarray(Wo.T).astype(BF16),
        "cosT": cosT,
        "tanT": tanT,
        "RT": RT,
        "mask2": mask2,
    }
    in_maps = []
    for c in range(NCORES):
        xc = x[c * BPC:(c + 1) * BPC]
        xT = np.ascontiguousarray(xc.transpose(2, 0, 1).reshape(D, T)).astype(BF16)
        in_maps.append({"xT": xT, **shared})
    return in_maps


def assemble(results):
    out = np.empty((B, S, D), np.float32)
    for c in range(NCORES):
        oT = np.asarray(results[c]["outT"])
        out[c * BPC:(c + 1) * BPC] = oT.reshape(D, BPC, S).transpose(1, 2, 0)
    return out


def run(x, Wq, Wk, Wv, Wo, trace=False, **run_kwargs):
    from concourse.bass_utils import run_bass_kernel_spmd
    nc = _get_nc()
    in_maps = make_in_maps(x, Wq, Wk, Wv, Wo)
    res = run_bass_kernel_spmd(
        nc, in_maps, core_ids=list(range(NCORES)), trace=trace, **run_kwargs)
    return assemble(res.results), res


def kernel(x, Wq, Wk, Wv, Wo):
    out, _ = run(np.asarray(x), np.asarray(Wq), np.asarray(Wk),
                 np.asarray(Wv), np.asarray(Wo))
    return out


# revision 50
# speedup vs baseline: 1.3894x; 1.3894x over previous
"""Multi-head attention (RoPE + causal softmax) Trainium2 Bass kernel.

Problem: nn_MultiHeadAttention (B=16, S=512, D=1024, H=16, Hd=64).
Sharding: data-parallel over batch — 2 batches per core on 8 NeuronCores.
Feature-major device layout; per-core pipeline: q/k projections + RoPE,
v projection (ones-column augmented), per-head-pair causal attention with
PE-packed score matmuls, softmax via exp + ones-row denominators, Wo
projection.  Measured ~244-246us on HW.
"""

import numpy as np
import ml_dtypes

BF16 = ml_dtypes.bfloat16

B, S, D = 16, 512, 1024
H, HD = 16, 64
NCORES = 8
BPC = B // NCORES
T = BPC * S

_CACHE = {}


def _rope_tables():
    inv_freq = 1.0 / (10000.0 ** (np.arange(0, HD, 2, dtype=np.float64) / HD))
    t = np.arange(S, dtype=np.float64)
    freqs = np.outer(t, inv_freq)
    emb = np.concatenate([freqs, freqs], -1)
    return np.cos(emb), np.sin(emb)


def _host_consts():
    cos, sin = _rope_tables()
    tan = sin / cos
    cols = np.arange(T) % S
    cosT = np.ascontiguousarray(np.tile(cos[cols].T, (2, 1))).astype(BF16)
    tanT = np.ascontiguousarray(np.tile(tan[cols].T, (2, 1))).astype(BF16)
    R64 = np.zeros((64, 64), np.float32)
    R64[np.arange(32), np.arange(32) + 32] = -1.0
    R64[np.arange(32) + 32, np.arange(32)] = 1.0
    R128 = np.zeros((128, 128), np.float32)
    R128[:64, :64] = R64
    R128[64:, 64:] = R64
    RT = np.ascontiguousarray(R128.T).astype(BF16)
    mask01 = (np.arange(128)[None, :] >= np.arange(128)[:, None]).astype(BF16)
    mask2 = np.ascontiguousarray(np.concatenate([mask01, mask01], axis=1))
    return cosT, tanT, RT, mask2


def _build_bass(dump_debug=False):
    import concourse.bacc as bacc
    import concourse.tile as tile
    import concourse.mybir as mybir

    dt = mybir.dt
    f32, bf16 = dt.float32, dt.bfloat16
    Exp = mybir.ActivationFunctionType.Exp

    nc = bacc.Bacc("TRN2", target_bir_lowering=False, debug=False, enable_asserts=False)

    xT_d = nc.dram_tensor("xT", [D, T], bf16, kind="ExternalInput").ap()
    wq_d = nc.dram_tensor("WqT", [D, D], bf16, kind="ExternalInput").ap()
    wk_d = nc.dram_tensor("WkT", [D, D], bf16, kind="ExternalInput").ap()
    wv_d = nc.dram_tensor("WvT", [D, D], bf16, kind="ExternalInput").ap()
    wo_d = nc.dram_tensor("WoT", [D, D], bf16, kind="ExternalInput").ap()
    cos_d = nc.dram_tensor("cosT", [128, T], bf16, kind="ExternalInput").ap()
    tan_d = nc.dram_tensor("tanT", [128, T], bf16, kind="ExternalInput").ap()
    rt_d = nc.dram_tensor("RT", [128, 128], bf16, kind="ExternalInput").ap()
    mask_d = nc.dram_tensor("mask2", [128, 256], bf16, kind="ExternalInput").ap()
    out_d = nc.dram_tensor("outT", [D, T], f32, kind="ExternalOutput").ap()

    KC = D // 128

    with tile.TileContext(nc) as tc:
        with (
            tc.tile_pool(name="consts", bufs=1) as consts,
            tc.tile_pool(name="persist", bufs=1) as persist,
            tc.tile_pool(name="work", bufs=3) as work,
            tc.tile_pool(name="expp", bufs=3) as expp,
            tc.tile_pool(name="ps_a", bufs=4, space="PSUM") as ps_a,
            tc.tile_pool(name="ps_b", bufs=2, space="PSUM") as ps_b,
        ):
            def load(pool, dram, shape, dtyp, nm):
                t_ = pool.tile(shape, dtyp, name=nm)
                nc.sync.dma_start(out=t_, in_=dram)
                return t_

            xT = [load(consts, xT_d[k * 128:(k + 1) * 128, :], [128, T], bf16, f"xT{k}") for k in range(KC)]
            wq = [load(consts, wq_d[k * 128:(k + 1) * 128, :], [128, D], bf16, f"wq{k}") for k in range(KC)]
            RT = load(consts, rt_d, [128, 128], bf16, "RT")
            cosT = load(consts, cos_d, [128, T], bf16, "cosT")
            tanT = load(consts, tan_d, [128, T], bf16, "tanT")
            wk = [load(consts, wk_d[k * 128:(k + 1) * 128, :], [128, D], bf16, f"wk{k}") for k in range(KC)]
            wv = [load(consts, wv_d[k * 128:(k + 1) * 128, :], [128, D], bf16, f"wv{k}") for k in range(KC)]
            mask2 = load(consts, mask_d, [128, 2, 128], bf16, "mask2")
            wo = [load(consts, wo_d[k * 128:(k + 1) * 128, :], [128, D], bf16, f"wo{k}") for k in range(KC)]

            qrot = [persist.tile([128, T], bf16, name=f"qrot{m}") for m in range(KC)]
            krot = [persist.tile([128, T], bf16, name=f"krot{m}") for m in range(KC)]
            vsb = [persist.tile([128, H * 65], bf16, name=f"vsb{t_}") for t_ in range(T // 128)]
            att = [persist.tile([128, T], bf16, name=f"att{m}") for m in range(KC)]

            for t_ in range(T // 128):
                vt = vsb[t_].rearrange("p (h w) -> p h w", w=65)
                nc.gpsimd.memset(vt[:, :, 64:65], 1.0)

            # qk projection with the RoPE tan trick, split into A (projection
            # matmuls + pre2 = pp*tan on DVE) and B (rotation matmul that
            # accumulates into the same PSUM + rot = pp*cos).  Emitting B
            # after the sibling A hides the pre2 chain behind 8 matmuls.
            # Identity: R@(pp*tan)*cos == rotate_half(pp)*sin because the
            # RoPE tables repeat with period 32.
            def emit_qk_A(nb, w_sb, rot, m):
                cols = slice(nb * S, (nb + 1) * S)
                pp = ps_a.tile([128, S], f32, name="pp", tag="ps_a")
                for k in range(KC):
                    nc.tensor.matmul(
                        pp, w_sb[k][:, m * 128:(m + 1) * 128], xT[k][:, cols],
                        start=(k == 0), stop=(k == KC - 1))
                pre2 = work.tile([128, S], bf16, name="pre2", tag="pre2", bufs=2)
                nc.vector.tensor_mul(pre2, pp, tanT[:, cols])
                return (pp, pre2, rot, m, cols)

            def emit_qk_B(st):
                pp, pre2, rot, m, cols = st
                nc.tensor.matmul(pp, RT, pre2, start=False, stop=True,
                                 skip_group_check=True)
                nc.vector.tensor_mul(rot[m][:, cols], pp, cosT[:, cols])

            def emit_qk_unit(nb, m):
                st_q = emit_qk_A(nb, wq, qrot, m)
                st_k = emit_qk_A(nb, wk, krot, m)
                emit_qk_B(st_q)
                emit_qk_B(st_k)

            def emit_v_group(b, tch, nh):
                vt = vsb[tch].rearrange("p (h w) -> p h w", w=65)
                vp = ps_a.tile([128, S], f32, name="vp", tag="ps_a")
                for k in range(KC):
                    nc.tensor.matmul(
                        vp, xT[k][:, tch * 128:(tch + 1) * 128],
                        wv[k][:, nh * S:(nh + 1) * S],
                        start=(k == 0), stop=(k == KC - 1))
                nc.scalar.copy(
                    vt[:, nh * 8:(nh + 1) * 8, 0:64],
                    vp.rearrange("p (h w) -> p h w", w=64))

            def emit_attn_head(b, h, exs):
                bcols = slice(b * S, (b + 1) * S)
                mh, p0 = h // 2, (h % 2) * 64
                hi = h % 2
                av = ps_a.tile([128, S], f32, name="av", tag="ps_a")
                for i in range(4):
                    lo = i * 128
                    nc.tensor.matmul(
                        av[0:65, lo:S],
                        vsb[b * 4 + i][:, h * 65: h * 65 + 65],
                        exs[i][:, hi, lo:S],
                        start=(i == 0), stop=(i == 3), skip_group_check=True)
                ss = work.tile([1, S], f32, name="ss", tag="ss")
                nc.vector.tensor_copy(ss, av[64:65, :])
                st = work.tile([128, 4], f32, name="st", tag="st")
                nc.gpsimd.dma_start(out=st, in_=ss)
                rt = work.tile([128, 4], f32, name="rt", tag="rt")
                nc.vector.reciprocal(rt, st)
                rr = work.tile([1, S], f32, name="rr", tag="rr")
                nc.gpsimd.dma_start(out=rr, in_=rt)
                rb = work.tile([64, S], f32, name="rb", tag="rb", bufs=2)
                nc.gpsimd.partition_broadcast(rb, rr)
                nc.vector.tensor_mul(att[mh][p0:p0 + 64, bcols], av[0:64, :], rb)

            def emit_attn_pair(b, j):
                mh = j
                exs = []
                for i in range(4):
                    lo = i * 128
                    sc = ps_b.tile([128, 2, S], f32, name="sc", tag="ps_b")
                    for hi, p0 in ((0, 0), (1, 64)):
                        nc.tensor.matmul(
                            sc[:, hi, 0:S - lo],
                            krot[mh][p0:p0 + 64, b * S + lo: b * S + lo + 128],
                            qrot[mh][p0:p0 + 64, b * S + lo: (b + 1) * S],
                            start=True, stop=True)
                    ex = expp.tile([128, 2, S], bf16, name="ex", tag=f"ex{i}")
                    nc.scalar.activation(ex[:, :, lo:S], sc[:, :, 0:S - lo], Exp, scale=0.125)
                    nc.vector.tensor_mul(ex[:, :, lo:lo + 128], ex[:, :, lo:lo + 128], mask2)
                    exs.append(ex)
                emit_attn_head(b, 2 * j, exs)
                emit_attn_head(b, 2 * j + 1, exs)

            def emit_wo_group(b, m):
                bcols = slice(b * S, (b + 1) * S)
                fin = ps_a.tile([128, S], f32, name="fin", tag="ps_a")
                for k in range(KC):
                    nc.tensor.matmul(
                        fin, wo[k][:, m * 128:(m + 1) * 128], att[k][:, bcols],
                        start=(k == 0), stop=(k == KC - 1))
                ob = work.tile([128, S], f32, name="ob", tag="ob", bufs=2)
                nc.vector.tensor_copy(ob, fin)
                nc.sync.dma_start(out=out_d[m * 128:(m + 1) * 128, bcols], in_=ob)

            for m in range(KC):
                emit_qk_unit(0, m)
            for tch in range(4):
                for nh in range(2):
                    emit_v_group(0, tch, nh)
            v1 = [(tch, nh) for tch in range(4, 8) for nh in range(2)]
            for j in range(H // 2):
                emit_attn_pair(0, j)
                emit_qk_unit(1, j)
                emit_v_group(1, *v1[j])
            for j in range(H // 2):
                emit_attn_pair(1, j)
                emit_wo_group(0, j)
            for m in range(KC):
                emit_wo_group(1, m)

    nc.compile()
    return nc


def _get_nc():
    if "nc" not in _CACHE:
        _CACHE["nc"] = _build_bass()
    return _CACHE["nc"]


def make_in_maps(x, Wq, Wk, Wv, Wo):
    cosT, tanT, RT, mask2 = _host_consts()
    shared = {
        "WqT": np.ascontiguousarray(Wq.T).astype(BF16),
        "WkT": np.ascontiguousarray(Wk.T).astype(BF16),
        "WvT": np.ascontiguousarray(Wv.T).astype(BF16),
        "WoT": np.ascontiguousarray(Wo.T).astype(BF16),
        "cosT": cosT,
        "tanT": tanT,
        "RT": RT,
        "mask2": mask2,
    }
    in_maps = []
    for c in range(NCORES):
        xc = x[c * BPC:(c + 1) * BPC]
        xT = np.ascontiguousarray(xc.transpose(2, 0, 1).reshape(D, T)).astype(BF16)
        in_maps.append({"xT": xT, **shared})
    return in_maps


def assemble(results):
    out = np.empty((B, S, D), np.float32)
    for c in range(NCORES):
        oT = np.asarray(results[c]["outT"])
        out[c * BPC:(c + 1) * BPC] = oT.reshape(D, BPC, S).transpose(1, 2, 0)
    return out


def run(x, Wq, Wk, Wv, Wo, trace=False, **run_kwargs):
    from concourse.bass_utils import run_bass_kernel_spmd
    nc = _get_nc()
    in_maps = make_in_maps(x, Wq, Wk, Wv, Wo)
    res = run_bass_kernel_spmd(
        nc, in_maps, core_ids=list(range(NCORES)), trace=trace, **run_kwargs)
    return assemble(res.results), res


def kernel(x, Wq, Wk, Wv, Wo):
    out, _ = run(np.asarray(x), np.asarray(Wq), np.asarray(Wk),
                 np.asarray(Wv), np.asarray(Wo))
    return out
